# revision 8
# baseline (speedup 1.0000x reference)
"""Correlation layer (avgpool2x2 + all-pairs view correlation) for Trainium2.

Reference computation (hardcoded shapes):
  x: (6, 512, 90, 90) fp32, n=3 views, b=2 samples.
  xp = avgpool2x2(x)                      -> (6, 512, 45, 45)
  xf = xp.reshape(2, 3, 512, 2025)
  for each sample, for the 6 ordered view pairs (i, j), i != j:
      corr[k, q, p] = sum_c xf[i, c, q] * xf[j, c, p]
  out: (12, 2025, 45, 45) fp32

Sharding over 8 cores: core = (b, pair-group g, q-half h).
  - b in {0,1}: sample.
  - g in {0,1}: pair group.  The device program always computes the cyclic
    pairs [(0,1), (1,2), (2,0)] over its 3 input views; g=1 cores receive the
    views permuted [0,2,1] so those program pairs are the anti-cyclic actual
    pairs [(0,2), (2,1), (1,0)].
  - h in {0,1}: which half of the q axis (pooled rows 0:23 vs 23:45).  h=1
    cores receive the raw rows rolled by -46 so that their q-half lands at
    pooled rows 0:23 of the (rolled) pooled grid; the host un-rolls the p
    axis of their output.

Each core: DMA in its 3 raw views as fp16 (24.9 MB), avg-pool on DVE (2 ops/tile),
store pooled features as float32r (TF32-like, full-rate matmul), run
3 pairs x 9 q-tiles x 4 cgroups x 4 n-chunks matmuls on PE, scale by 1/16
during PSUM->SBUF eviction on ACT, DMA out (3, 1035, 2025) fp32.
"""

import numpy as np

_NC = None

# Program pair list (cyclic) and the actual reference-pair index k each
# program pair maps to, per pair-group g.  Reference order:
# [(0,1),(0,2),(1,0),(1,2),(2,0),(2,1)] -> k = 0..5
_PAIRS = [(0, 1), (1, 2), (2, 0)]
_KMAP = [[0, 3, 4], [1, 5, 2]]

_QROWS = 23          # pooled rows per core's q-half (h=1 only uses 22)
_Q = _QROWS * 45     # 1035
_QT = 9              # q tiles: 9 x 128 (last one only has 11 valid rows)
# float32r matmuls need an even moving-dim, so pad 2025 -> 2026 with a zero col
_NCHUNK = [512, 512, 512, 490]


def _build_nc(reps=None):
    """Build the per-core program.  reps: if set, wrap the whole body in an
    on-device For_i loop executing it `reps` times (used only for timing)."""
    from contextlib import nullcontext

    from concourse import bacc
    import concourse.mybir as mybir
    from concourse.tile import TileContext

    f32 = mybir.dt.float32
    f16 = mybir.dt.float16
    f32r = mybir.dt.float32r

    nc = bacc.Bacc("TRN2", target_bir_lowering=False, debug=False, num_devices=8)
    x = nc.dram_tensor("x", (3, 4, 128, 8100), f16, kind="ExternalInput")
    out = nc.dram_tensor("out", (3, _Q, 2025), f32, kind="ExternalOutput")

    with TileContext(nc) as tc:
        with (
            tc.tile_pool(name="fpool", bufs=1) as fpool,
            tc.tile_pool(name="stage", bufs=3) as stage,
            tc.tile_pool(name="opool", bufs=3) as opool,
            tc.tile_pool(name="psum", bufs=2, space="PSUM") as psum,
        ):
            # Persistent pooled features, rounded to float32r for the PE.
            # Column 2025 is a zero pad (f32r matmul needs even moving-dim).
            F = [
                [fpool.tile([128, 2026], f32r, tag=f"F_{v}_{g}", name=f"F_{v}_{g}") for g in range(4)]
                for v in range(3)
            ]
            for v in range(3):
                for g in range(4):
                    nc.vector.memset(F[v][g][:, 2025:2026].bitcast(f32), 0.0)

            loop = (
                tc.For_i(
                    0, reps, 1,
                    hint_engines=(
                        mybir.EngineType.PE,
                        mybir.EngineType.SP,
                        mybir.EngineType.Activation,
                        mybir.EngineType.DVE,
                    ),
                )
                if reps is not None
                else nullcontext()
            )
            with loop:
                # --- avg-pool 2x2 (sums; the /16 is applied at eviction) ---
                for v in range(3):
                    for g in range(4):
                        for ch in range(3):  # chunks of 30 raw rows
                            raw = stage.tile([128, 2700], f16, tag="raw", name="raw")
                            nc.sync.dma_start(
                                raw[:], x[v, g, :, ch * 2700 : (ch + 1) * 2700]
                            )
                            rv = raw[:].rearrange(
                                "p (r two w) -> p r two w", two=2, w=90
                            )
                            t1 = stage.tile([128, 1350], f32, tag="t1", name="t1")
                            nc.vector.tensor_tensor(
                                out=t1[:].rearrange("p (r w) -> p r w", w=90),
                                in0=rv[:, :, 0],
                                in1=rv[:, :, 1],
                                op=mybir.AluOpType.add,
                            )
                            with nc.allow_low_precision(reason="f32r pooled features"):
                                nc.vector.reduce_sum(
                                    out=F[v][g][:, ch * 675 : (ch + 1) * 675],
                                    in_=t1[:].rearrange("p (a two) -> p a two", two=2),
                                    axis=mybir.AxisListType.X,
                                )

                # --- correlation matmuls ---
                for pi, (a, b) in enumerate(_PAIRS):
                    for qt in range(_QT):
                        q0 = qt * 128
                        qs = min(128, _Q - q0)  # valid output rows (11 on last)
                        pt = psum.tile([128, 2048], f32, tag="pt", name="pt")
                        for g in range(4):
                            n0 = 0
                            for ns in _NCHUNK:
                                # f32r matmul requires all column groups active:
                                # always run M=128 (F has >=1152 columns).
                                nc.tensor.matmul(
                                    pt[:, n0 : n0 + ns],
                                    lhsT=F[a][g][:, q0 : q0 + 128],
                                    rhs=F[b][g][:, n0 : n0 + ns],
                                    start=(g == 0),
                                    stop=(g == 3),
                                )
                                n0 += ns
                        ot = opool.tile([128, 2025], f32, tag="ot", name="ot")
                        nc.scalar.mul(ot[:qs], pt[:qs, :2025], 1.0 / 16.0)
                        nc.sync.dma_start(out[pi, q0 : q0 + qs, :], ot[:qs])

    nc.finalize()
    return nc


def _core_inputs(x):
    """Per-core pre-permuted raw input, shaped (3, 4, 128, 8100) fp32."""
    ins = []
    for c in range(8):
        b, g, h = c // 4, (c // 2) % 2, c % 2
        xb = x[b * 3 : (b + 1) * 3]
        if g:
            xb = xb[[0, 2, 1]]
        if h:
            xb = np.roll(xb, -46, axis=2)
        ins.append(
            {"x": np.ascontiguousarray(xb, dtype=np.float16).reshape(3, 4, 128, 8100)}
        )
    return ins


def _gather(results):
    """Assemble the 8 per-core outputs into the full (12, 2025, 45, 45)."""
    out = np.empty((12, 45, 45, 45, 45), dtype=np.float32)
    for c in range(8):
        b, g, h = c // 4, (c // 2) % 2, c % 2
        oc = results[c]["out"].reshape(3, _QROWS, 45, 45, 45)
        if h:
            oc = np.roll(oc[:, :22], 23, axis=3)
            qrows = slice(23, 45)
        else:
            oc = oc[:, :23]
            qrows = slice(0, 23)
        for pi in range(3):
            k = _KMAP[g][pi]
            out[b * 6 + k, qrows] = oc[pi]
    return out.reshape(12, 2025, 45, 45)


def kernel(x, n):
    global _NC
    x = np.asarray(x, dtype=np.float32)
    assert int(n) == 3 and x.shape == (6, 512, 90, 90), (x.shape, n)
    from concourse.bass_utils import run_bass_kernel_spmd

    if _NC is None:
        _NC = _build_nc()
    res = run_bass_kernel_spmd(_NC, _core_inputs(x), core_ids=list(range(8)))
    return _gather(res.results)


# revision 11
# speedup vs baseline: 12.2950x; 12.2950x over previous
"""Correlation layer (avgpool2x2 + all-pairs view correlation) for Trainium2.

Reference computation (hardcoded shapes):
  x: (6, 512, 90, 90) fp32, n=3 views, b=2 samples.
  xp = avgpool2x2(x)                      -> (6, 512, 45, 45)
  xf = xp.reshape(2, 3, 512, 2025)
  for each sample, for the 6 ordered view pairs (i, j), i != j:
      corr[k, q, p] = sum_c xf[i, c, q] * xf[j, c, p]
  out: (12, 2025, 45, 45) fp32

Sharding over 8 cores: core = (b, pair-group g, q-half h).
  - b in {0,1}: sample.
  - g in {0,1}: pair group.  The device program always computes the cyclic
    pairs [(0,1), (1,2), (2,0)] over its 3 input views; g=1 cores receive the
    views permuted [0,2,1] so those program pairs are the anti-cyclic actual
    pairs [(0,2), (2,1), (1,0)].
  - h in {0,1}: which half of the q axis (pooled rows 0:23 vs 23:45).  h=1
    cores receive the raw rows rolled by -46 so that their q-half lands at
    pooled rows 0:23 of the (rolled) pooled grid; the host un-rolls the p
    axis of their output.

Each core: DMA in its 3 raw views as fp16 (24.9 MB), avg-pool on DVE (2 ops/tile),
store pooled features as float32r (TF32-like, full-rate matmul), run
3 pairs x 9 q-tiles x 4 cgroups x 4 n-chunks matmuls on PE, scale by 1/16
during PSUM->SBUF eviction on ACT, DMA out (3, 1035, 2025) fp32.
"""

import numpy as np

_NC = None

# Program pair list (cyclic) and the actual reference-pair index k each
# program pair maps to, per pair-group g.  Reference order:
# [(0,1),(0,2),(1,0),(1,2),(2,0),(2,1)] -> k = 0..5
_PAIRS = [(0, 1), (1, 2), (2, 0)]
_KMAP = [[0, 3, 4], [1, 5, 2]]

_QROWS = 23          # pooled rows per core's q-half (h=1 only uses 22)
_Q = _QROWS * 45     # 1035
_QT = 9              # q tiles: 9 x 128 (last one only has 11 valid rows)
# float32r matmuls need an even moving-dim, so pad 2025 -> 2026 with a zero col
_NCHUNK = [512, 512, 512, 490]


def _build_nc(reps=None):
    """Build the per-core program.  reps: if set, wrap the whole body in an
    on-device For_i loop executing it `reps` times (used only for timing)."""
    from contextlib import nullcontext

    from concourse import bacc
    import concourse.mybir as mybir
    from concourse.tile import TileContext

    f32 = mybir.dt.float32
    f16 = mybir.dt.float16
    f32r = mybir.dt.float32r

    nc = bacc.Bacc("TRN2", target_bir_lowering=False, debug=False, num_devices=8)
    x = nc.dram_tensor("x", (3, 4, 128, 8100), f16, kind="ExternalInput")
    out = nc.dram_tensor("out", (3, _Q, 2025), f32, kind="ExternalOutput")

    with TileContext(nc) as tc:
        with (
            tc.tile_pool(name="fpool", bufs=1) as fpool,
            tc.tile_pool(name="stage", bufs=3) as stage,
            tc.tile_pool(name="opool", bufs=3) as opool,
            tc.tile_pool(name="psum", bufs=2, space="PSUM") as psum,
        ):
            # Persistent pooled features, rounded to float32r for the PE.
            # Column 2025 is a zero pad (f32r matmul needs even moving-dim).
            F = [
                [fpool.tile([128, 2026], f32r, tag=f"F_{v}_{g}", name=f"F_{v}_{g}") for g in range(4)]
                for v in range(3)
            ]
            for v in range(3):
                for g in range(4):
                    nc.vector.memset(F[v][g][:, 2025:2026].bitcast(f32), 0.0)

            loop = (
                tc.For_i(
                    0, reps, 1,
                    hint_engines=(
                        mybir.EngineType.PE,
                        mybir.EngineType.SP,
                        mybir.EngineType.Activation,
                        mybir.EngineType.DVE,
                    ),
                )
                if reps is not None
                else nullcontext()
            )
            with loop:
                # --- avg-pool 2x2 (sums; the /16 is applied at eviction) ---
                for v in range(3):
                    for g in range(4):
                        # One big (2.07 MB) DMA per channel group.
                        raw = stage.tile([128, 8100], f16, tag="raw", name="raw")
                        nc.sync.dma_start(raw[:], x[v, g])
                        for ch in range(3):  # pool in chunks of 30 raw rows
                            rv = raw[:, ch * 2700 : (ch + 1) * 2700].rearrange(
                                "p (r two w) -> p r two w", two=2, w=90
                            )
                            t1 = stage.tile([128, 1350], f32, tag="t1", name="t1")
                            nc.vector.tensor_tensor(
                                out=t1[:].rearrange("p (r w) -> p r w", w=90),
                                in0=rv[:, :, 0],
                                in1=rv[:, :, 1],
                                op=mybir.AluOpType.add,
                            )
                            with nc.allow_low_precision(reason="f32r pooled features"):
                                nc.vector.reduce_sum(
                                    out=F[v][g][:, ch * 675 : (ch + 1) * 675],
                                    in_=t1[:].rearrange("p (a two) -> p a two", two=2),
                                    axis=mybir.AxisListType.X,
                                )

                # --- correlation matmuls ---
                for pi, (a, b) in enumerate(_PAIRS):
                    for qt2 in range(5):  # q-tile pairs: (0,1),(2,3),...,(8,)
                        tiles = [2 * qt2] + ([2 * qt2 + 1] if 2 * qt2 + 1 < _QT else [])
                        ot = opool.tile([128, len(tiles), 2025], f32, tag="ot", name="ot")
                        for t, qt in enumerate(tiles):
                            q0 = qt * 128
                            qs = min(128, _Q - q0)  # valid rows (11 on last)
                            pt = psum.tile([128, 2048], f32, tag="pt", name="pt")
                            for g in range(4):
                                n0 = 0
                                for ns in _NCHUNK:
                                    # f32r matmul requires all column groups
                                    # active: always run M=128.
                                    nc.tensor.matmul(
                                        pt[:, n0 : n0 + ns],
                                        lhsT=F[a][g][:, q0 : q0 + 128],
                                        rhs=F[b][g][:, n0 : n0 + ns],
                                        start=(g == 0),
                                        stop=(g == 3),
                                    )
                                    n0 += ns
                            nc.scalar.mul(ot[:qs, t, :], pt[:qs, :2025], 1.0 / 16.0)
                        # One store for the tile pair (2.07 MB).
                        q0 = 2 * qt2 * 128
                        rows = min(_Q - q0, len(tiles) * 128)
                        dst = out[pi, q0 : q0 + rows, :]
                        if rows == 256:
                            nc.sync.dma_start(
                                dst.rearrange("(t p) s -> p t s", p=128), ot[:]
                            )
                        else:
                            nc.sync.dma_start(dst, ot[:rows, 0, :])

    nc.finalize()
    return nc


def _core_inputs(x):
    """Per-core pre-permuted raw input, shaped (3, 4, 128, 8100) fp32."""
    ins = []
    for c in range(8):
        b, g, h = c // 4, (c // 2) % 2, c % 2
        xb = x[b * 3 : (b + 1) * 3]
        if g:
            xb = xb[[0, 2, 1]]
        if h:
            xb = np.roll(xb, -46, axis=2)
        ins.append(
            {"x": np.ascontiguousarray(xb, dtype=np.float16).reshape(3, 4, 128, 8100)}
        )
    return ins


def _gather(results):
    """Assemble the 8 per-core outputs into the full (12, 2025, 45, 45)."""
    out = np.empty((12, 45, 45, 45, 45), dtype=np.float32)
    for c in range(8):
        b, g, h = c // 4, (c // 2) % 2, c % 2
        oc = results[c]["out"].reshape(3, _QROWS, 45, 45, 45)
        if h:
            oc = np.roll(oc[:, :22], 23, axis=3)
            qrows = slice(23, 45)
        else:
            oc = oc[:, :23]
            qrows = slice(0, 23)
        for pi in range(3):
            k = _KMAP[g][pi]
            out[b * 6 + k, qrows] = oc[pi]
    return out.reshape(12, 2025, 45, 45)


def kernel(x, n):
    global _NC
    x = np.asarray(x, dtype=np.float32)
    assert int(n) == 3 and x.shape == (6, 512, 90, 90), (x.shape, n)
    from concourse.bass_utils import run_bass_kernel_spmd

    if _NC is None:
        _NC = _build_nc()
    res = run_bass_kernel_spmd(_NC, _core_inputs(x), core_ids=list(range(8)))
    return _gather(res.results)
